# revision 73
# baseline (speedup 1.0000x reference)
"""Trainium2 Bass kernel for a dense transformer block (nn_Block_31387620999284).

Sharding: 8 cores = 4 batches x 2 parity groups. Core c handles batch b=c//2
and the query tokens with sequence parity d=c%2 (positions d, d+2, ...). Every
core computes K/V for its batch's full 2048-token sequence (duplicated across
the pair), which removes all cross-core communication. Parity interleaving
makes the causal-attention work identical on every core, so a single NEFF runs
SPMD on all 8 cores with per-core input data only.

On-device layout is "transposed" throughout: [features on partitions, tokens on
free dim]. LayerNorm statistics are computed with ones-vector matmuls on the
tensor engine (partition-dim reduction) and replicated back across partitions
with K=1 matmuls. Matmuls run in bf16 (weights and LN inputs pre-cast on the
host) with fp32 PSUM accumulation; softmax skips the max-subtraction (scores
for this block are bounded by ~3, exp is safe). The softmax denominator rides
along as a 65th ones-column in V. The causal diagonal is handled by trimming
score/exp/AV columns per key tile; the parity geometry collapses the partial
mask to a single [128, 64] pattern. The attention-output projection is
interleaved into attention t=1 and the FFN first-matmul stream so the tensor
engine never drains. LN-affine params are folded into weights/biases on host.
"""

import sys

for _p in ("/opt/trn_rl_repo",):
    if _p not in sys.path:
        sys.path.append(_p)

import numpy as np
import ml_dtypes
from contextlib import ExitStack

import concourse.bass as bass
import concourse.tile as tile
from concourse import bacc, mybir
from concourse.bass import ts
from concourse.bass_utils import run_bass_kernel_spmd


def _install_ntff_hook():
    """The container's antenv stub lacks axon_hooks; provide it so tracing
    (BASS_TRACE=1) works instead of crashing on import."""
    try:
        import antenv.axon_hooks  # noqa: F401
        return
    except ImportError:
        pass
    try:
        import types
        import antenv
        mod = types.ModuleType("antenv.axon_hooks")
        mod._hook = None
        mod.set_axon_ntff_profile_hook = lambda h: setattr(mod, "_hook", h)
        mod.get_axon_ntff_profile_hook = lambda: mod._hook
        sys.modules["antenv.axon_hooks"] = mod
        antenv.axon_hooks = mod
        try:
            from trn_agent_boot.trn_boot import _ntff_profile_via_ctypes
            mod._hook = _ntff_profile_via_ctypes("/opt/axon/libaxon_pjrt.so")
        except Exception:
            pass
    except Exception:
        pass


_install_ntff_hook()

P = 128
D = 1024
TKV = 2048
TQ = 1024
F = 4096
H = 16
HD = 64
DP = D // P    # 8
FP = F // P    # 32
CH = 512       # token chunk / matmul free dim
QB = 512       # attention query block
NQB = TQ // QB # 2
NKT = TKV // P # 16 key tiles
EPS = 1e-5

F32 = mybir.dt.float32
BF16 = mybir.dt.bfloat16
AF = mybir.ActivationFunctionType
INTERLEAVE_WO = True


def build_nc():
    nc = bacc.Bacc("TRN2", target_bir_lowering=False, debug=False)

    xbT = nc.dram_tensor("xbT", [D, TKV], BF16, kind="ExternalInput").ap()
    xobT = nc.dram_tensor("xobT", [D, TQ], BF16, kind="ExternalInput").ap()
    xoT = nc.dram_tensor("xoT", [D, TQ], F32, kind="ExternalInput").ap()
    wq = nc.dram_tensor("wq", [D, D], BF16, kind="ExternalInput").ap()
    wk = nc.dram_tensor("wk", [D, D], BF16, kind="ExternalInput").ap()
    wv = nc.dram_tensor("wv", [D, D], BF16, kind="ExternalInput").ap()
    wo = nc.dram_tensor("wo", [D, D], BF16, kind="ExternalInput").ap()
    w1 = nc.dram_tensor("w1", [D, F], BF16, kind="ExternalInput").ap()
    w2 = nc.dram_tensor("w2", [F, D], BF16, kind="ExternalInput").ap()
    # bias columns: bo 0:8 | b2 8:16 | bq 16:24 | bk 24:32 | b1' 32:64
    biases = nc.dram_tensor("biases", [P, 64], F32, kind="ExternalInput").ap()
    bvr = nc.dram_tensor("bvr", [P, D], F32, kind="ExternalInput").ap()
    mk = nc.dram_tensor("mk", [P, 2, 64], BF16, kind="ExternalInput").ap()
    outT = nc.dram_tensor("outT", [D, TQ], F32, kind="ExternalOutput").ap()

    xb3 = xbT.rearrange("(o p) t -> p o t", p=P)
    xob3 = xobT.rearrange("(o p) t -> p o t", p=P)
    xoT3 = xoT.rearrange("(o p) t -> p o t", p=P)
    out3 = outT.rearrange("(o p) t -> p o t", p=P)
    wq3 = wq.rearrange("(o p) m -> p o m", p=P)
    wk3 = wk.rearrange("(o p) m -> p o m", p=P)
    wv3 = wv.rearrange("(o p) m -> p o m", p=P)
    wo3 = wo.rearrange("(o p) m -> p o m", p=P)
    w13 = w1.rearrange("(o p) m -> p o m", p=P)
    w23 = w2.rearrange("(o p) m -> p o m", p=P)

    with tile.TileContext(nc) as tc, ExitStack() as ctx:
        consts = ctx.enter_context(tc.tile_pool(name="consts", bufs=1))
        bias_sb = consts.tile([P, 64], F32, name="bias_sb")
        nc.sync.dma_start(bias_sb[:], biases)
        ones_b16 = consts.tile([P, 1], BF16, name="ones_b16")
        nc.vector.memset(ones_b16[:], 1.0)
        ones_f32 = consts.tile([1, P], F32, name="ones_f32")
        nc.vector.memset(ones_f32[:], 1.0)
        eps_sb = consts.tile([P, 1], F32, name="eps_sb")
        nc.vector.memset(eps_sb[:], EPS)

        # ---- LayerNorm (transposed layout) ----
        def make_ln_pools(stack, pfx):
            return dict(
                sq=stack.enter_context(tc.tile_pool(name=pfx + "sq", bufs=2)),
                st=stack.enter_context(tc.tile_pool(name=pfx + "st", bufs=2, space="PSUM")),
                sm=stack.enter_context(tc.tile_pool(name=pfx + "sm", bufs=1)),
                rep=stack.enter_context(tc.tile_pool(name=pfx + "rep", bufs=1, space="PSUM")),
                rsb=stack.enter_context(tc.tile_pool(name=pfx + "rsb", bufs=2)),
            )

        def ln_stats(lp, get_src, hc):
            """Stats half of LN: fills hc (bf16) if get_src given, computes the
            -mu and sd rows. Returns (r_mu, r_m2) row tiles."""
            ps_su = lp["st"].tile([1, CH], F32, name="ps_su", tag="st")
            ps_sq = lp["st"].tile([1, CH], F32, name="ps_sq", tag="st")
            for ks in range(DP):
                if get_src is not None:
                    nc.scalar.copy(hc[:, ks], get_src(ks))
                sq = lp["sq"].tile([P, CH], BF16, name="sq", tag="sq")
                nc.scalar.activation(sq[:], hc[:, ks], AF.Square)
                nc.tensor.matmul(ps_su[:], ones_b16[:], hc[:, ks],
                                 start=(ks == 0), stop=(ks == DP - 1))
                nc.tensor.matmul(ps_sq[:], ones_b16[:], sq[:],
                                 start=(ks == 0), stop=(ks == DP - 1))
            r_mu = lp["sm"].tile([1, CH], F32, name="r_mu", tag="r_mu")
            nc.vector.tensor_scalar_mul(r_mu[:], ps_su[:], -1.0 / D)
            r_m2 = lp["sm"].tile([1, CH], F32, name="r_m2", tag="r_m2")
            nc.vector.tensor_scalar_mul(r_m2[:], ps_sq[:], 1.0 / D)
            mu2 = lp["sm"].tile([1, CH], F32, name="mu2", tag="mu2")
            nc.vector.tensor_mul(mu2[:], r_mu[:], r_mu[:])
            nc.vector.tensor_sub(r_m2[:], r_m2[:], mu2[:])
            nc.scalar.activation(r_m2[:], r_m2[:], AF.Sqrt, bias=eps_sb[0:1])
            return r_mu, r_m2

        def ln_finish(lp, hc, rows):
            """Replicate rows across partitions (PE), then normalize hc."""
            r_mu, r_m2 = rows
            repS = lp["rep"].tile([P, CH], F32, name="repS", tag="repS")
            nc.tensor.matmul(repS[:], ones_f32[:], r_m2[:], start=True, stop=True)
            repM = lp["rep"].tile([P, CH], F32, name="repM", tag="repM")
            nc.tensor.matmul(repM[:], ones_f32[:], r_mu[:], start=True, stop=True)
            repA = lp["rsb"].tile([P, CH], F32, name="repA", tag="repA")
            nc.vector.reciprocal(repA[:], repS[:])
            repB = lp["rsb"].tile([P, CH], F32, name="repB", tag="repB")
            nc.vector.tensor_mul(repB[:], repM[:], repA[:])
            for ks in range(DP):
                nc.vector.tensor_mul(hc[:, ks], hc[:, ks], repA[:])
                nc.vector.tensor_add(hc[:, ks], hc[:, ks], repB[:])

        def ln_norm(lp, get_src, hc):
            ln_finish(lp, hc, ln_stats(lp, get_src, hc))

        # Long-lived right-side pools (fresh addresses -> DMAs never blocked
        # by address reuse of freed left-side pools).
        sWX = ExitStack()
        mskp = sWX.enter_context(tc.tile_pool(name="mskp", bufs=1, side="right"))
        mask_sb = mskp.tile([P, 2, 64], BF16, name="mask_sb")
        nc.sync.dma_start(mask_sb[:], mk)
        wop = sWX.enter_context(tc.tile_pool(name="wop", bufs=1, side="right"))
        wo_sb = wop.tile([P, DP, D], BF16, name="wo_sb")
        xop = sWX.enter_context(tc.tile_pool(name="xop", bufs=3, side="right"))

        # Persistent K/V/Q for attention (phases 1-2).
        sKVQ = ExitStack()
        kvqp = sKVQ.enter_context(tc.tile_pool(name="kvqp", bufs=1))
        KT_all = kvqp.tile([P, DP, TKV], BF16, name="KT_all")
        V_all = kvqp.tile([P, NKT, H, HD + 1], BF16, name="V_all")
        QT_all = kvqp.tile([P, DP, TQ], BF16, name="QT_all")

        # ================= Phase 1: LN1 + Q/K/V projections =================
        with ExitStack() as p1:
            lp1 = make_ln_pools(p1, "l1")
            hcp = p1.enter_context(tc.tile_pool(name="hcp", bufs=4))
            mmp = p1.enter_context(tc.tile_pool(name="mmp1", bufs=3, space="PSUM"))
            bvp = p1.enter_context(tc.tile_pool(name="bvp", bufs=1))
            wkvp = p1.enter_context(tc.tile_pool(name="wkvp", bufs=1))
            wqp = p1.enter_context(tc.tile_pool(name="wqp", bufs=1, side="right"))

            # chunk list: 4 KV chunks then 2 Q chunks, software-pipelined:
            # chunk i+1's stats run during chunk i's first matmul groups, its
            # replicate+normalize is traced mid-way so the PE never waits on
            # the DVE/ACT stat chain.
            chunks = [("kv", c) for c in range(TKV // CH)] + \
                     [("q", c) for c in range(TQ // CH)]
            hcs = {}

            def ln_start(idx):
                kind, c = chunks[idx]
                src3 = xb3 if kind == "kv" else xob3
                hc = hcp.tile([P, DP, CH], BF16, name="hc", tag="hc")
                nc.sync.dma_start(hc[:], src3[:, :, ts(c, CH)])
                ps_su = lp1["st"].tile([1, CH], F32, name="ps_su", tag="st")
                ps_sq = lp1["st"].tile([1, CH], F32, name="ps_sq", tag="st")
                hcs[idx] = [hc, (ps_su, ps_sq), None]

            def ln_step(idx, ks):
                hc, (ps_su, ps_sq), _ = hcs[idx]
                sq = lp1["sq"].tile([P, CH], BF16, name="sq", tag="sq")
                nc.scalar.activation(sq[:], hc[:, ks], AF.Square)
                nc.tensor.matmul(ps_su[:], ones_b16[:], hc[:, ks],
                                 start=(ks == 0), stop=(ks == DP - 1))
                nc.tensor.matmul(ps_sq[:], ones_b16[:], sq[:],
                                 start=(ks == 0), stop=(ks == DP - 1))

            def ln_rows(idx):
                hc, (ps_su, ps_sq), _ = hcs[idx]
                lp = lp1
                r_mu = lp["sm"].tile([1, CH], F32, name="r_mu", tag="r_mu")
                nc.vector.tensor_scalar_mul(r_mu[:], ps_su[:], -1.0 / D)
                r_m2 = lp["sm"].tile([1, CH], F32, name="r_m2", tag="r_m2")
                nc.vector.tensor_scalar_mul(r_m2[:], ps_sq[:], 1.0 / D)
                mu2 = lp["sm"].tile([1, CH], F32, name="mu2", tag="mu2")
                nc.vector.tensor_mul(mu2[:], r_mu[:], r_mu[:])
                nc.vector.tensor_sub(r_m2[:], r_m2[:], mu2[:])
                nc.scalar.activation(r_m2[:], r_m2[:], AF.Sqrt, bias=eps_sb[0:1])
                hcs[idx][2] = (r_mu, r_m2)

            def ln_end(idx):
                hc, _, rows = hcs[idx]
                ln_finish(lp1, hc, rows)
                return hc

            # prologue: chunk 0's LN runs un-pipelined
            ln_start(0)
            for ks in range(DP):
                ln_step(0, ks)
            ln_rows(0)

            # bulk weight loads on the gpsimd DMA queue so they don't delay
            # the per-chunk x streams on the sync queue
            nc.vector.memset(V_all[:, :, :, HD:HD + 1], 1.0)
            bvr_sb = bvp.tile([P, D], F32, name="bvr_sb")
            nc.gpsimd.dma_start(bvr_sb[:], bvr)
            wk_sb = wkvp.tile([P, DP, D], BF16, name="wk_sb")
            nc.gpsimd.dma_start(wk_sb[:], wk3)
            wv_sb = wkvp.tile([P, DP, D], BF16, name="wv_sb")
            nc.gpsimd.dma_start(wv_sb[:], wv3)
            wq_sb = wqp.tile([P, DP, D], BF16, name="wq_sb")
            nc.gpsimd.dma_start(wq_sb[:], wq3)
            nc.gpsimd.dma_start(wo_sb[:], wo3)

            for idx, (kind, c) in enumerate(chunks):
                hc = ln_end(idx) if idx == 0 else hcs.pop(idx)[0]
                nxt = idx + 1 < len(chunks)
                if kind == "kv":
                    for hp in range(DP):
                        if hp == 0 and nxt:
                            ln_start(idx + 1)
                        ps = mmp.tile([P, CH], F32, name="psk", tag="mm1")
                        for ks in range(DP):
                            nc.tensor.matmul(ps[:], wk_sb[:, ks, ts(hp, P)], hc[:, ks],
                                             start=(ks == 0), stop=(ks == DP - 1))
                        nc.vector.tensor_scalar_add(KT_all[:, hp, ts(c, CH)], ps[:],
                                                    bias_sb[:, 24 + hp:25 + hp])
                        if nxt:
                            ln_step(idx + 1, hp)
                    if nxt:
                        ln_rows(idx + 1)
                    for dc in range(2):
                        for st in range(4):
                            ps = mmp.tile([P, CH], F32, name="psv", tag="mm1")
                            for ks in range(DP):
                                nc.tensor.matmul(ps[:], hc[:, ks, ts(st, P)],
                                                 wv_sb[:, ks, ts(dc, CH)],
                                                 start=(ks == 0), stop=(ks == DP - 1))
                            vdst = V_all[:, c * 4 + st, dc * 8:dc * 8 + 8, 0:HD]
                            nc.vector.tensor_add(
                                vdst,
                                ps[:].rearrange("p (h d) -> p h d", h=8),
                                bvr_sb[:, ts(dc, CH)].rearrange("p (h d) -> p h d", h=8))
                        if dc == 0 and nxt:
                            ln_end(idx + 1)
                else:
                    for hp in range(DP):
                        if hp == 0 and nxt:
                            ln_start(idx + 1)
                        ps = mmp.tile([P, CH], F32, name="psq", tag="mm1")
                        for ks in range(DP):
                            nc.tensor.matmul(ps[:], wq_sb[:, ks, ts(hp, P)], hc[:, ks],
                                             start=(ks == 0), stop=(ks == DP - 1))
                        nc.vector.tensor_scalar_add(QT_all[:, hp, ts(c, CH)], ps[:],
                                                    bias_sb[:, 16 + hp:17 + hp])
                        if nxt:
                            ln_step(idx + 1, hp)
                    if nxt:
                        ln_rows(idx + 1)
                        ln_end(idx + 1)

        # ============ Phase 2+3: attention with interleaved out-proj ============
        sX2 = ExitStack()
        x2p = sX2.enter_context(tc.tile_pool(name="x2p", bufs=1, side="right"))
        x2T = x2p.tile([P, DP, TQ], F32, name="x2T")
        h2T = x2p.tile([P, DP, TQ], BF16, name="h2T")
        sATT = ExitStack()
        attp = sATT.enter_context(tc.tile_pool(name="attp", bufs=1, side="right"))
        attn_all = attp.tile([P, DP, TQ], BF16, name="attn_all")
        p3t = ExitStack()       # tp3 outlives p2 pools (used by wo epilogues)
        tp3 = p3t.enter_context(tc.tile_pool(name="tp3", bufs=4, side="right"))

        def wo_group(qc, i, pspool, pstag):
            """One out-projection output tile + residual epilogue -> x2T."""
            ps = pspool.tile([P, CH], F32, name="pso", tag=pstag)
            for ks in range(DP):
                nc.tensor.matmul(ps[:], wo_sb[:, ks, ts(i, P)],
                                 attn_all[:, ks, ts(qc, CH)],
                                 start=(ks == 0), stop=(ks == DP - 1))
            xo = xop.tile([P, CH], F32, name="xo", tag="xo")
            nc.sync.dma_start(xo[:], xoT3[:, i, ts(qc, CH)])
            t1 = tp3.tile([P, CH], F32, name="t1", tag="t1")
            nc.vector.tensor_add(t1[:], ps[:], xo[:])
            nc.vector.tensor_scalar_add(x2T[:, i, ts(qc, CH)], t1[:],
                                        bias_sb[:, 0 + i:1 + i])

        with ExitStack() as p2:
            psS = p2.enter_context(tc.tile_pool(name="psS", bufs=2, space="PSUM"))
            psAV = p2.enter_context(tc.tile_pool(name="psAV", bufs=3, space="PSUM"))
            psWO = p2.enter_context(tc.tile_pool(name="psWO", bufs=1, space="PSUM"))
            weip = p2.enter_context(tc.tile_pool(name="weip", bufs=4))
            smal = p2.enter_context(tc.tile_pool(name="smal", bufs=4))

            scale = float(HD) ** -0.5
            pend = []  # deferred softmax-denominator sections

            def flush_den():
                for t_, hp_, pavs_ in pend:
                    for l in range(2):
                        pb = 64 * l
                        pav = pavs_[l]
                        den = smal.tile([1, QB], F32, name="den", tag="den")
                        nc.scalar.copy(den[:], pav[64:65, :])
                        nc.tensor.matmul(pav[64:128, :], ones_f32[:, 0:64],
                                         den[:], start=True, stop=True)
                        rec = smal.tile([64, QB], F32, name="rec", tag="rec")
                        nc.vector.reciprocal(rec[:], pav[64:128, :])
                        nc.vector.tensor_mul(attn_all[pb:pb + 64, hp_, ts(t_, QB)],
                                             pav[0:64, :], rec[:])
                pend.clear()

            for t in range(NQB):
                nkt = 8 * (t + 1)
                for hp in range(DP):
                    # drain the previous head-pair's denominator sections before
                    # grabbing fresh accumulator tiles
                    flush_den()
                    if t == 1 and INTERLEAVE_WO:
                        # t=0 results are complete: inject one out-projection
                        # group to keep the PE stream dense.
                        wo_group(0, hp, psWO, "pswo")
                    pavs = [psAV.tile([P, QB], F32, name=f"pav{l}", tag="pav")
                            for l in range(2)]
                    weis = {}

                    # Key tile kt = 8*t + j is "diagonal": columns < 64*j are
                    # fully masked (skipped), [64j, 64j+64) get the stride-2
                    # mask, the rest are fully allowed.
                    def col0(kt, t=t):
                        return 64 * (kt - 8 * t) if kt >= 8 * t else 0

                    def scores2(kt, t=t, hp=hp, weis=weis):
                        o = col0(kt)
                        ps2 = psS.tile([P, 2, QB], F32, name="pss2", tag="pss2")
                        for l in range(2):
                            pb = 64 * l
                            nc.tensor.matmul(ps2[:, l, o:],
                                             KT_all[pb:pb + 64, hp, ts(kt, P)],
                                             QT_all[pb:pb + 64, hp,
                                                    t * QB + o:(t + 1) * QB],
                                             start=True, stop=True)
                        wei2 = weip.tile([P, 2, QB], BF16, name="wei2", tag="wei2")
                        nc.scalar.activation(wei2[:, :, o:], ps2[:, :, o:], AF.Exp,
                                             scale=scale)
                        if kt >= 8 * t:
                            nc.vector.tensor_mul(wei2[:, :, o:o + 64],
                                                 wei2[:, :, o:o + 64], mask_sb[:])
                        weis[kt] = wei2

                    scores2(0)
                    scores2(1)
                    for kt in range(nkt):
                        if kt + 2 < nkt:
                            scores2(kt + 2)
                        o = col0(kt)
                        wei2 = weis.pop(kt)
                        for l in range(2):
                            nc.tensor.matmul(pavs[l][0:65, o:],
                                             V_all[:, kt, 2 * hp + l, :],
                                             wei2[:, l, o:],
                                             start=(kt == 0), stop=(kt == nkt - 1))
                    pend.append((t, hp, pavs))
            flush_den()
            if not INTERLEAVE_WO:
                for i in range(DP):
                    wo_group(0, i, psWO, "pswo")
        sKVQ.close()

        # ================= Phase 3b/4: LN2 + FFN + residual =================
        with ExitStack() as p34:
            lp3 = make_ln_pools(p34, "l3")
            rp = p34.enter_context(tc.tile_pool(name="rp", bufs=1))
            top = p34.enter_context(tc.tile_pool(name="top", bufs=4))
            w1p = p34.enter_context(tc.tile_pool(name="w1p", bufs=2, side="right"))
            w2p = p34.enter_context(tc.tile_pool(name="w2p", bufs=2, side="right"))
            with ExitStack() as p4ps:
                psF = p4ps.enter_context(tc.tile_pool(name="psF", bufs=2, space="PSUM"))
                psO = p4ps.enter_context(tc.tile_pool(name="psO", bufs=2, space="PSUM"))

                for i in range(DP):
                    wo_group(1, i, psF, "psf")
                ln_norm(lp3, lambda ks: x2T[:, ks, ts(0, CH)], h2T[:, :, ts(0, CH)])
                for qc in range(TQ // CH):
                    rT = rp.tile([P, FP, CH], BF16, name="rT", tag="rT")
                    for fs in range(8):
                        w1c = w1p.tile([P, DP, CH], BF16, name="w1c", tag="w1c")
                        nc.sync.dma_start(w1c[:], w13[:, :, ts(fs, CH)])
                        for fj in range(4):
                            f = fs * 4 + fj
                            ps = psF.tile([P, CH], F32, name="psf", tag="psf")
                            for ks in range(DP):
                                nc.tensor.matmul(ps[:], w1c[:, ks, ts(fj, P)],
                                                 h2T[:, ks, ts(qc, CH)],
                                                 start=(ks == 0), stop=(ks == DP - 1))
                            nc.scalar.activation(rT[:, f], ps[:], AF.Relu,
                                                 bias=bias_sb[:, 32 + f:33 + f])
                    if qc == 0:
                        ln_norm(lp3, lambda ks: x2T[:, ks, ts(1, CH)],
                                h2T[:, :, ts(1, CH)])
                    for i in range(DP):
                        w2i = w2p.tile([P, FP, P], BF16, name="w2i", tag="w2i")
                        nc.sync.dma_start(w2i[:], w23[:, :, ts(i, P)])
                        ps2 = psO.tile([P, CH], F32, name="ps2", tag="ps2")
                        for f in range(FP):
                            nc.tensor.matmul(ps2[:], w2i[:, f, :], rT[:, f],
                                             start=(f == 0), stop=(f == FP - 1))
                        t2 = top.tile([P, CH], F32, name="t2", tag="t2")
                        nc.vector.tensor_add(t2[:], ps2[:], x2T[:, i, ts(qc, CH)])
                        ot = top.tile([P, CH], F32, name="ot", tag="ot")
                        nc.vector.tensor_scalar_add(ot[:], t2[:],
                                                    bias_sb[:, 8 + i:9 + i])
                        nc.sync.dma_start(out3[:, i, ts(qc, CH)], ot[:])
        p3t.close()
        sATT.close()
        sX2.close()
        sWX.close()

    nc.compile()
    return nc


def prepare_inputs(x, wq, wk, wv, wo, bo, w1, b1, w2, b2,
                   g_ln1, b_ln1, g_ln2, b_ln2):
    """Host-side sharding/prep. Returns list of 8 per-core input dicts."""
    f32 = np.float32
    bf = ml_dtypes.bfloat16
    x = np.asarray(x, f32)
    g1 = np.asarray(g_ln1, f32)
    b1n = np.asarray(b_ln1, f32)
    g2 = np.asarray(g_ln2, f32)
    b2n = np.asarray(b_ln2, f32)

    wq_e = np.ascontiguousarray((g1[:, None] * np.asarray(wq, f32)).astype(bf))
    wk_e = np.ascontiguousarray((g1[:, None] * np.asarray(wk, f32)).astype(bf))
    wv_e = np.ascontiguousarray((g1[:, None] * np.asarray(wv, f32)).astype(bf))
    wo_e = np.ascontiguousarray(np.asarray(wo, f32).astype(bf))
    w1_e = np.ascontiguousarray((g2[:, None] * np.asarray(w1, f32)).astype(bf))
    w2_e = np.ascontiguousarray(np.asarray(w2, f32).astype(bf))

    bq = b1n @ np.asarray(wq, f32)
    bk = b1n @ np.asarray(wk, f32)
    bv = b1n @ np.asarray(wv, f32)
    b1p = np.asarray(b1, f32) + b2n @ np.asarray(w1, f32)

    def pcol(v, n):  # [n*128] -> [128, n] partition-major
        return np.ascontiguousarray(np.asarray(v, f32).reshape(n, P).T)

    biases = np.zeros((P, 64), f32)
    biases[:, 0:8] = pcol(bo, 8)
    biases[:, 8:16] = pcol(b2, 8)
    biases[:, 16:24] = pcol(bq, 8)
    biases[:, 24:32] = pcol(bk, 8)
    biases[:, 32:64] = pcol(b1p, 32)
    bvr = np.ascontiguousarray(np.broadcast_to(bv[None, :], (P, D)))

    masks = {}
    for d in (0, 1):
        p = np.arange(P)[:, None]
        r = np.arange(64)[None, :]
        m = (p <= (2 * r + d)).astype(bf)
        masks[d] = np.ascontiguousarray(np.broadcast_to(m[:, None, :], (P, 2, 64)))

    in_maps = []
    for c in range(8):
        b, d = divmod(c, 2)
        xT = x[b].T
        xo = x[b, d::2].T
        in_maps.append(dict(
            xbT=np.ascontiguousarray(xT.astype(bf)),
            xobT=np.ascontiguousarray(xo.astype(bf)),
            xoT=np.ascontiguousarray(xo),
            wq=wq_e, wk=wk_e, wv=wv_e, wo=wo_e, w1=w1_e, w2=w2_e,
            biases=biases, bvr=bvr, mk=masks[d],
        ))
    return in_maps


_NC = None
LAST_RESULTS = None


def kernel(**inputs):
    global _NC, LAST_RESULTS
    in_maps = prepare_inputs(**inputs)
    if _NC is None:
        _NC = build_nc()
    res = run_bass_kernel_spmd(_NC, in_maps, core_ids=list(range(8)))
    LAST_RESULTS = res
    out = np.empty((4, TKV, D), np.float32)
    for c in range(8):
        b, d = divmod(c, 2)
        out[b, d::2, :] = res.results[c]["outT"].T
    return out


if __name__ == "__main__":
    z = np.load("/root/problem/ref_cache.npz")
    inputs = {k: z[k] for k in z.files if k != "out"}
    out = kernel(**inputs)
    ref = z["out"]
    err = np.abs(out - ref)
    print("abs max err:", err.max(), "scale-rel:", err.max() / np.abs(ref).max())
